# revision 15
# baseline (speedup 1.0000x reference)
"""Multi-head attention (B=4, S=2048, D=1024, H=16) on 8 Trainium2 cores.

Sharding: core c = (batch b = c//2, head-group g = c%2). Each core computes
8 heads' attention for one batch element plus the partial output projection
for its head-group's rows of Wo; the host sums the two partials per batch
and adds the bias.

Per-core kernel (all matmuls bf16, fp32 accumulation). The Scalar engine's
exp stream (256 x [128,1024], ~1us each) is the bottleneck; everything else
is paced to keep it saturated:
  xT      = host-transposed bf16 x, DMA'd in seq-column stripes  [D, S]
  qT, kT  = Wg.T @ x.T per 512-col chunk (later pairs' chunks    [G, S]
            trickle into earlier pairs' attention as PE filler)
  vpo     = [x @ Wv | 1] per head (ones col fused)               [S, 8, 65]
  per pair, per key-block kb (scores lead ctx by 2 blocks):
    sT    = k_h @ q_h.T (row-paired heads, K=64)         PSUM [128, 1024]
    pT    = exp(sT / 8) on ScalarE -> bf16 SBUF
    cacc += [v_h|1].T @ pT  (K=128, M=65)                PSUM [65, 512]
            row 64 accumulates the softmax denominator for free
  normalization: cacc is staged to SBUF at once (frees the PSUM bank for
  the next pair), then DVE recip -> GpSimd partition_broadcast -> DVE
  mult run off the critical path.
  out     = ctxT.T @ Wo_g per chunk, trickled into the next q-chunk's
            attention -> fp32 partial to DRAM             [S, D]
"""

import numpy as np

B, S, D = 4, 2048, 1024
H, HD = 16, 64
NCORES = 8
G = D // 2  # head-group width per core (8 heads x 64)

_BUILD_CACHE = {}


def build_mha(S=S, D=D, G=G, HD=HD):
    """Build the per-core Bass program. Returns the Bass object."""
    key = (S, D, G, HD)
    if key in _BUILD_CACHE:
        return _BUILD_CACHE[key]

    import concourse.bacc as bacc
    import concourse.mybir as mybir
    import concourse.tile as tile
    from contextlib import ExitStack

    FP32 = mybir.dt.float32
    BF16 = mybir.dt.bfloat16

    P = 128
    DC = D // P          # d_in chunks
    GC = G // P          # head-pair chunks
    SB = S // P          # seq blocks
    W = 512              # q-chunk width: keeps ctx accumulators at 2 PSUM
    NW = S // W          # banks so the score pool can triple-buffer
    NH = G // HD         # heads per core (8)
    LEAD = 2             # ctx trails scores/exp by LEAD key blocks
    assert G % P == 0 and HD == 64 and S % 512 == 0

    nc = bacc.Bacc("TRN2", target_bir_lowering=False, debug=False)
    xt_d = nc.declare_dram_parameter("xt", [D, S], BF16, isOutput=False)
    wq_d = nc.declare_dram_parameter("wq", [D, G], BF16, isOutput=False)
    wk_d = nc.declare_dram_parameter("wk", [D, G], BF16, isOutput=False)
    wv_d = nc.declare_dram_parameter("wv", [D, G], BF16, isOutput=False)
    wo_d = nc.declare_dram_parameter("wo", [G, D], BF16, isOutput=False)
    out_d = nc.declare_dram_parameter("out", [S, D], FP32, isOutput=True)

    with tile.TileContext(nc) as tc, ExitStack() as ctx:
        const = ctx.enter_context(tc.tile_pool(name="const", bufs=1))
        wpool = ctx.enter_context(tc.tile_pool(name="wpool", bufs=1))
        big = ctx.enter_context(tc.tile_pool(name="big", bufs=1))
        ppool = ctx.enter_context(tc.tile_pool(name="ppool", bufs=6))
        sgp = ctx.enter_context(tc.tile_pool(name="sgp", bufs=4))
        bcp = ctx.enter_context(tc.tile_pool(name="bcp", bufs=4))
        dpool = ctx.enter_context(tc.tile_pool(name="dpool", bufs=4))
        outp = ctx.enter_context(tc.tile_pool(name="outp", bufs=6))
        pscore = ctx.enter_context(tc.tile_pool(name="pscore", bufs=3, space="PSUM"))
        pctx = ctx.enter_context(tc.tile_pool(name="pctx", bufs=2, space="PSUM"))

        zbias = const.tile([P, 1], FP32)
        nc.gpsimd.memset(zbias[:], 0.0)

        # ---- loads: weights first, then xT in seq-column stripes so the
        # first projection chunks can start after ~1/4 of the x load ----
        wq_sb = wpool.tile([P, DC, G], BF16)
        wk_sb = wpool.tile([P, DC, G], BF16)
        wv_sb = wpool.tile([P, DC, G], BF16)
        wo_sb = wpool.tile([P, GC, D], BF16)
        xt = big.tile([P, DC, S], BF16)

        def dma_w(w_d, w_sb, nch):
            for c in range(nch):
                nc.sync.dma_start(w_sb[:, c, :], w_d[c * P:(c + 1) * P, :])

        def dma_x_stripe(sc):
            for dc in range(DC):
                nc.sync.dma_start(
                    xt[:, dc, sc * 512:(sc + 1) * 512],
                    xt_d[dc * P:(dc + 1) * P, sc * 512:(sc + 1) * 512],
                )

        # Q/K weights and the first x stripes land first so the pair-0
        # projection (and with it the exp stream) starts ~15us in
        dma_w(wq_d, wq_sb, DC)
        dma_w(wk_d, wk_sb, DC)
        dma_x_stripe(0)
        dma_x_stripe(1)
        dma_w(wv_d, wv_sb, DC)
        dma_x_stripe(2)
        dma_x_stripe(3)
        dma_w(wo_d, wo_sb, GC)

        # ---- projections ----
        qt = big.tile([P, GC, S], BF16)
        kt = big.tile([P, GC, S], BF16)
        # V with a fused ones column per head: [128 keys, kb, head, 64+1]
        vpo = big.tile([P, SB, NH, HD + 1], BF16)
        # ones columns (offset 64 of each head slot)
        nc.gpsimd.memset(vpo[:, :, :, HD:HD + 1], 1.0)

        # Chunk emitters yield one instruction per step: their matmuls get
        # interleaved between attention matmuls so the PSUM accumulation
        # chains (8-deep, serially dependent) hide each other's write-read
        # bubbles. 1024-wide chunks halve the matmul/ldweights count.
        def gen_qk_chunk(g, w_sb, dst, sc):
            ps = pscore.tile([P, 512], FP32, tag="pscore", name="ps")
            for dc in range(DC):
                nc.tensor.matmul(
                    ps[:],
                    lhsT=w_sb[:, dc, g * P:(g + 1) * P],
                    rhs=xt[:, dc, sc * 512:(sc + 1) * 512],
                    start=(dc == 0),
                    stop=(dc == DC - 1),
                )
                yield
            nc.vector.tensor_copy(dst[:, g, sc * 512:(sc + 1) * 512], ps[:])
            yield

        def qk_gens(g):
            for sc in range(S // 512):
                for w_sb, dst in ((wq_sb, qt), (wk_sb, kt)):
                    yield gen_qk_chunk(g, w_sb, dst, sc)

        def gen_proj_v(sb):
            ps = pscore.tile([P, G], FP32, tag="pscore", name="ps")
            for dc in range(DC):
                nc.tensor.matmul(
                    ps[:],
                    lhsT=xt[:, dc, sb * P:(sb + 1) * P],
                    rhs=wv_sb[:, dc, :],
                    start=(dc == 0),
                    stop=(dc == DC - 1),
                )
                yield
            # scatter the 8 heads' 64-wide slices into the 65-strided layout
            nc.vector.tensor_copy(vpo[:, sb, :, 0:HD], ps[:])
            yield

        from collections import deque
        ops = deque()   # pending single-instruction emission steps

        def pop_op():
            while ops:
                try:
                    next(ops[0])
                    return
                except StopIteration:
                    ops.popleft()

        def drain_ops():
            while ops:
                try:
                    next(ops[0])
                except StopIteration:
                    ops.popleft()

        # pair 0's Q/K for the first 512 keys/queries, then the first V
        # blocks; the rest feed the interleaver inside pair 0's key loop,
        # ordered against their use-deadlines (kt chunk sc is read from
        # key block 4*sc on; V block j from the ctx emitted at step j+2).
        for w_sb, dst in ((wq_sb, qt), (wk_sb, kt)):
            for _ in gen_qk_chunk(0, w_sb, dst, 0):
                pass
        VPRE = 4
        for sb in range(min(VPRE, SB)):
            for _ in gen_proj_v(sb):
                pass
        ops.append(gen_proj_v(4))
        ops.append(gen_proj_v(5))
        ops.append(gen_qk_chunk(0, wq_sb, qt, 1))
        ops.append(gen_qk_chunk(0, wk_sb, kt, 1))
        ops.append(gen_proj_v(6))
        ops.append(gen_proj_v(7))
        ops.append(gen_qk_chunk(0, wq_sb, qt, 2))
        ops.append(gen_qk_chunk(0, wk_sb, kt, 2))
        ops.append(gen_proj_v(8))
        ops.append(gen_proj_v(9))
        ops.append(gen_proj_v(10))
        ops.append(gen_qk_chunk(0, wq_sb, qt, 3))
        ops.append(gen_qk_chunk(0, wk_sb, kt, 3))
        for sb in range(11, SB):
            ops.append(gen_proj_v(sb))

        # ---- attention + normalization + output projection ----
        ctxT = big.tile([P, GC, S], BF16)
        EXP = mybir.ActivationFunctionType.Exp
        scale = float(1.0 / np.sqrt(HD))

        for qw in range(NW):
            q0 = qw * W
            for p in range(GC):
                if qw == 0:
                    # this pair's Q/K must be fully emitted before its
                    # scores read qt/kt (pacing guard; normally a no-op
                    # given the static op counts)
                    drain_ops()
                    if p + 1 < GC:
                        ops.extend(qk_gens(p + 1))
                hA, hB = 2 * p, 2 * p + 1
                # per-head accumulators: rows 0..63 ctx, row 64 denominator
                cacc = [
                    pctx.tile([HD + 1, 512], FP32, tag="pctx", name=f"cacc{h}")
                    for h in range(2)
                ]

                def emit_scores_exp_chunk(kb):
                    # one PSUM tile per q-chunk holding BOTH heads [A | B]:
                    # the key-block's row-paired score matmuls feed a single
                    # exp, and the triple-buffered pool lets the PE run up
                    # to three key blocks ahead of the Scalar engine.
                    qs = q0
                    s = pscore.tile([P, 1024], FP32, tag="pscore", name="s")
                    nc.tensor.matmul(
                        s[:, 0:512],
                        lhsT=kt[0:64, p, kb * P:(kb + 1) * P],
                        rhs=qt[0:64, p, qs:qs + 512],
                        start=True, stop=True,
                    )
                    nc.tensor.matmul(
                        s[:, 512:1024],
                        lhsT=kt[64:128, p, kb * P:(kb + 1) * P],
                        rhs=qt[64:128, p, qs:qs + 512],
                        start=True, stop=True,
                    )
                    pt = ppool.tile([P, 1024], BF16, tag="ppool", name="pt")
                    nc.scalar.activation(
                        pt[:], s[:], EXP, bias=zbias[:], scale=scale
                    )
                    return pt

                def emit_ctx_chunk(kb, pt):
                    # one filler step between the two heads' matmuls hides
                    # the accumulators' PSUM write-read bubble
                    first, last = kb == 0, kb == SB - 1
                    for h, hh in ((0, hA), (1, hB)):
                        nc.tensor.matmul(
                            cacc[h][:],
                            lhsT=vpo[:, kb, hh % NH, :],
                            rhs=pt[:, h * 512:(h + 1) * 512],
                            start=first, stop=last,
                        )
                        pop_op()

                # software pipeline: ctx trails scores/exp by LEAD key
                # blocks, with PE filler interleaved between attention
                # matmuls to keep the exp stream fed and chain bubbles full.
                burst = 7 if (qw == 0 and p == 0) else 1
                pts = {}
                for kb in range(SB):
                    pts[kb] = emit_scores_exp_chunk(kb)
                    pop_op()
                    if kb >= LEAD:
                        emit_ctx_chunk(kb - LEAD, pts.pop(kb - LEAD))
                    pop_op()
                    for _ in range(burst):
                        pop_op()
                if qw == 0 and p == 0:
                    drain_ops()  # tail ctx reads the last V blocks
                for kb in range(SB - LEAD, SB):
                    emit_ctx_chunk(kb, pts.pop(kb))

                # stage ctx+den to SBUF at once: frees the PSUM bank ~1us
                # after the last ctx matmul; the rest of the normalization
                # (recip -> broadcast -> mult) runs off the critical path.
                qs = q0
                for h in range(2):
                    sg = sgp.tile([HD + 1, 512], FP32, tag="sgp", name="sg")
                    nc.vector.tensor_copy(sg[:], cacc[h][:])
                    # custom-DVE recip misreads nonzero partition offsets on
                    # HW; bounce the den row through a partition-0 tile
                    dc_ = dpool.tile([1, 512], FP32, tag="dpool", name="denc")
                    nc.vector.tensor_copy(dc_[0:1, :], sg[HD:HD + 1, :])
                    dt_ = dpool.tile([1, 512], FP32, tag="dpool", name="den")
                    nc.vector.reciprocal_approx_fast(
                        dt_[0:1, :], dc_[0:1, :]
                    )
                    bc = bcp.tile([HD, 512], FP32, tag="bcp", name="bc")
                    nc.gpsimd.partition_broadcast(bc[:], dt_[0:1, :])
                    nc.vector.tensor_tensor(
                        ctxT[h * HD:(h + 1) * HD, p, qs:qs + 512],
                        sg[0:HD, :],
                        bc[:],
                        mybir.AluOpType.mult,
                    )

            # output projection for this q chunk: queued as filler for the
            # next q chunk's attention; the final chunk is the kernel tail
            def gen_out_chunk(row, nck):
                po = pscore.tile([P, 512], FP32, tag="pscore", name="po")
                for g in range(GC):
                    nc.tensor.matmul(
                        po[:],
                        lhsT=ctxT[:, g, row:row + P],
                        rhs=wo_sb[:, g, nck * 512:(nck + 1) * 512],
                        start=(g == 0),
                        stop=(g == GC - 1),
                    )
                    yield
                ob = outp.tile([P, 512], FP32, tag="ob")
                nc.vector.tensor_copy(ob[:], po[:])
                nc.sync.dma_start(
                    out_d[row:row + P, nck * 512:(nck + 1) * 512], ob[:]
                )
                yield

            rcs = [
                (q0 + sb * P, nck)
                for sb in range(W // P)
                for nck in range(D // 512)
            ]
            if qw == NW - 1:
                drain_ops()
                for row, nck in rcs:
                    for _ in gen_out_chunk(row, nck):
                        pass
            else:
                ops.extend(gen_out_chunk(row, nck) for row, nck in rcs)

    nc.compile()
    _BUILD_CACHE[key] = nc
    return nc


def make_shards(x, Wq, Wk, Wv, Wo):
    """Split full inputs into 8 per-core input maps.

    Host-side layout prep only (dtype narrowing + transpose): the kernel
    consumes bf16 and x with the model dim on partitions.
    """
    import ml_dtypes
    BF = ml_dtypes.bfloat16
    x = np.asarray(x, dtype=np.float32)
    xt = np.ascontiguousarray(x.transpose(0, 2, 1)).astype(BF)  # [B, D, S]
    Wqb = np.asarray(Wq, dtype=np.float32).astype(BF)
    Wkb = np.asarray(Wk, dtype=np.float32).astype(BF)
    Wvb = np.asarray(Wv, dtype=np.float32).astype(BF)
    Wob = np.asarray(Wo, dtype=np.float32).astype(BF)
    shards = []
    for c in range(NCORES):
        b, g = divmod(c, 2)
        cs = slice(g * G, (g + 1) * G)
        shards.append({
            "xt": xt[b],
            "wq": np.ascontiguousarray(Wqb[:, cs]),
            "wk": np.ascontiguousarray(Wkb[:, cs]),
            "wv": np.ascontiguousarray(Wvb[:, cs]),
            "wo": np.ascontiguousarray(Wob[cs, :]),
        })
    return shards


def combine(results, bo):
    """Sum head-group partials per batch and add bias."""
    bo = np.asarray(bo, dtype=np.float32)
    outs = [results[c]["out"] for c in range(NCORES)]
    return np.stack([outs[2 * b] + outs[2 * b + 1] for b in range(B)]) + bo


def run_shards(shards, trace=False, **kw):
    from concourse.bass_utils import run_bass_kernel_spmd
    nc = build_mha()
    return run_bass_kernel_spmd(nc, shards, list(range(NCORES)), trace=trace, **kw)


def kernel(x, Wq, Wk, Wv, Wo, bo):
    res = run_shards(make_shards(x, Wq, Wk, Wv, Wo))
    return combine(res.results, bo)
